# revision 10
# baseline (speedup 1.0000x reference)
"""LSTM encoder (B=128, T=2048, I=64, H=128) on 8 Trainium2 NeuronCores.

Strategy: data-parallel over batch (16 rows/core). Each core runs the full
sequential recurrence with a transposed layout ([H, b] tiles) so the gate
matmuls use the full 128-partition contraction and the elementwise phase
uses all 128 lanes.

Math (per step, PyTorch gate order i,f,g,o):
    gates = x W_ih^T + b_ih + b_hh + h W_hh^T
    c' = sigmoid(f) c + sigmoid(i) tanh(g);  h' = sigmoid(o) tanh(c')

On-chip algebra uses tanh only, with sigma(x) = (tanh(x/2)+1)/2 folded into
weight prescaling. State kept as S = 2c, H2 = 2h:
    PSUM slot s in {i,f,o}: a_s/2 ; slot g: a_g     (a_* = true preactivations)
    t* = tanh(PSUM)                                  -> ACT1, one instr
    u  = (t_i + 1) * t_g                             -> DVE stt
    t1 = (t_f + 1) * S                               -> DVE stt (fused w/ u)
    S' = 0.5 t1 + u                                  -> DVE stt
    y  = tanh(0.5 S')                                -> ACT2
    H2' = (t_o + 1) * y                              -> DVE stt
The input projections for a 16-step chunk are matmul'd into PSUM first
(start=True); the per-step recurrence matmuls accumulate onto them
(start=False), so no separate gate add is needed.
"""

import numpy as np
from contextlib import ExitStack

import concourse.bass as bass
import concourse.tile as tile
from concourse import bacc, mybir
from concourse.bass_utils import run_bass_kernel_spmd

F32 = mybir.dt.float32
BF16 = mybir.dt.bfloat16
AF = mybir.ActivationFunctionType
OP = mybir.AluOpType

B, T_FULL, I_DIM, H = 128, 2048, 64, 128
NCORES = 8
BL = B // NCORES            # 16 batch rows per core

# PSUM slot order (i, f, o, g); index into PyTorch gate order (i, f, g, o)
GATE_PERM = [0, 1, 3, 2]
GATE_SCALE = [0.5, 0.5, 0.5, 1.0]

CHUNK = 16                  # timesteps of input projection per PSUM chunk
HSG = 8                     # steps per hs staging group (PE transpose granularity)
OUTG = 64                   # steps per output DMA


def _build(T):
    assert T % OUTG == 0 and T % CHUNK == 0
    nchunks = T // CHUNK
    nc = bacc.Bacc("TRN2", target_bir_lowering=False, debug=False,
                   num_devices=NCORES)

    x_d = nc.dram_tensor("x", [BL, T, I_DIM], F32, kind="ExternalInput")
    h2_d = nc.dram_tensor("h2", [BL, H], F32, kind="ExternalInput")   # 2*h0
    s_d = nc.dram_tensor("s", [BL, H], F32, kind="ExternalInput")     # 2*c0
    rw_d = nc.dram_tensor("rw", [H, 4 * H], BF16, kind="ExternalInput")
    wi_d = nc.dram_tensor("wi", [I_DIM + 1, 4 * H], F32, kind="ExternalInput")
    id_d = nc.dram_tensor("ident", [128, 128], F32, kind="ExternalInput")
    hs_d = nc.dram_tensor("hs", [BL, T, H], F32, kind="ExternalOutput")

    with tile.TileContext(nc) as tc, ExitStack() as ctx:
        const = ctx.enter_context(tc.tile_pool(name="const", bufs=1))
        state = ctx.enter_context(tc.tile_pool(name="state", bufs=1))
        xap = ctx.enter_context(tc.tile_pool(name="xa", bufs=1))
        h2p = ctx.enter_context(tc.tile_pool(name="h2b", bufs=2))
        scrp = ctx.enter_context(tc.tile_pool(name="scr", bufs=2))
        hsp = ctx.enter_context(tc.tile_pool(name="hsstg", bufs=2))
        outp = ctx.enter_context(tc.tile_pool(name="outstg", bufs=3))
        xinp = ctx.enter_context(tc.tile_pool(name="xin", bufs=3))
        pgp = ctx.enter_context(tc.tile_pool(name="pg", bufs=2, space="PSUM"))
        ptp = ctx.enter_context(tc.tile_pool(name="pt", bufs=2, space="PSUM"))

        rw_sb = const.tile([H, 4 * H], BF16)
        wi_sb = const.tile([I_DIM + 1, 4 * H], F32)
        ident = const.tile([128, 128], F32)
        nc.sync.dma_start(rw_sb[:], rw_d[:])
        nc.sync.dma_start(wi_sb[:], wi_d[:])
        nc.sync.dma_start(ident[:], id_d[:])

        # act_s columns: 0:64 tanh gates (i,f,o,g), 64:80 S state, 80:96 y
        act_s = state.tile([128, 96], F32)
        # Two persistent x-hat buffers [I+1, CHUNK*BL]; row 64 = ones (bias).
        xa_bufs = [xap.tile([I_DIM + 1, CHUNK * BL], F32, tag=f"xa{i}",
                            name=f"xa{i}")
                   for i in range(2)]
        for xb in xa_bufs:
            nc.vector.memset(xb[I_DIM:I_DIM + 1, :], 1.0)

        # ---- initial state ----
        init_in = xinp.tile([128, 128], F32, tag="init")
        nc.sync.dma_start(init_in[0:BL, :], h2_d[:])
        init_in2 = xinp.tile([128, 128], F32, tag="init")
        nc.sync.dma_start(init_in2[0:BL, :], s_d[:])
        pt0 = ptp.tile([128, 128], F32, tag="pt")
        nc.tensor.transpose(pt0[:, 0:BL], init_in[0:BL, :], ident[0:BL, 0:BL])
        h2b_prev = h2p.tile([128, BL], BF16)
        nc.vector.tensor_copy(h2b_prev[:], pt0[:, 0:BL])
        pt1 = ptp.tile([128, 128], F32, tag="pt")
        nc.tensor.transpose(pt1[:, 0:BL], init_in2[0:BL, :], ident[0:BL, 0:BL])
        nc.vector.tensor_copy(act_s[:, 64:80], pt1[:, 0:BL])

        def emit_xhat(ch, half):
            """Load+transpose 8 steps of x into xa buffer for chunk ch."""
            t0 = ch * CHUNK + half * 8
            xa = xa_bufs[ch % 2]
            xin = xinp.tile([128, I_DIM], F32)
            src = x_d[:, t0:t0 + 8, :].rearrange("b t i -> t b i")
            nc.sync.dma_start(xin[:], src)
            pt = ptp.tile([128, 128], F32, tag="pt")
            nc.tensor.transpose(pt[0:I_DIM, :], xin[:], ident[:])
            nc.vector.tensor_copy(
                xa[0:I_DIM, half * 128:(half + 1) * 128], pt[0:I_DIM, 0:128])

        def emit_xg(ch, pg, half):
            """Input-projection matmuls for 8 steps into PSUM chunk."""
            xa = xa_bufs[ch % 2]
            v = pg[:].rearrange("p (t s b) -> p t s b", t=CHUNK, s=4)
            for s in range(4):
                # start=True clears the has_written bits of the WHOLE bank,
                # so only the first matmul into each bank may set it.
                nc.tensor.matmul(
                    v[:, half * 8:(half + 1) * 8, s, :],
                    wi_sb[:, s * H:(s + 1) * H],
                    xa[:, half * 128:(half + 1) * 128],
                    start=(s == 0), stop=False, skip_group_check=True)

        # chunk 0 production
        pg_cur = pgp.tile([128, CHUNK * 64], F32, tag="pg")
        emit_xhat(0, 0)
        emit_xhat(0, 1)
        emit_xg(0, pg_cur, 0)
        emit_xg(0, pg_cur, 1)
        pg_next = None

        hs_cur = None
        out_cur = None
        for t in range(T):
            ch, tl = divmod(t, CHUNK)
            # interleave next-chunk production
            if ch + 1 < nchunks:
                if tl == 0:
                    pg_next = pgp.tile([128, CHUNK * 64], F32, tag="pg")
                    emit_xhat(ch + 1, 0)
                elif tl == 2:
                    emit_xhat(ch + 1, 1)
                elif tl == 8:
                    emit_xg(ch + 1, pg_next, 0)
                elif tl == 10:
                    emit_xg(ch + 1, pg_next, 1)

            # ---- recurrence step ----
            vg = pg_cur[:, tl * 64:(tl + 1) * 64]
            for s in range(4):
                nc.tensor.matmul(
                    vg[:, s * BL:(s + 1) * BL],
                    rw_sb[:, s * H:(s + 1) * H],
                    h2b_prev[:],
                    start=False, stop=True, skip_group_check=True)
            nc.scalar.activation(act_s[:, 0:64], vg[:], AF.Tanh)
            scr = scrp.tile([128, 2 * BL], F32)
            # [u | t1] = (([t_i|t_f]) + 1) * ([t_g | S])
            nc.vector.scalar_tensor_tensor(
                scr[:], act_s[:, 0:32], 1.0, act_s[:, 48:80], OP.add, OP.mult)
            # S' = 0.5*t1 + u
            nc.vector.scalar_tensor_tensor(
                act_s[:, 64:80], scr[:, BL:2 * BL], 0.5, scr[:, 0:BL],
                OP.mult, OP.add)
            nc.scalar.activation(act_s[:, 80:96], act_s[:, 64:80], AF.Tanh,
                                 scale=0.5)
            h2b_new = h2p.tile([128, BL], BF16)
            nc.vector.scalar_tensor_tensor(
                h2b_new[:], act_s[:, 32:48], 1.0, act_s[:, 80:96],
                OP.add, OP.mult)
            if t % HSG == 0:
                hs_cur = hsp.tile([128, HSG * BL], F32)
            nc.vector.scalar_tensor_tensor(
                hs_cur[:, (t % HSG) * BL:(t % HSG + 1) * BL],
                act_s[:, 32:48], 1.0, act_s[:, 80:96], OP.add, OP.mult)
            h2b_prev = h2b_new

            if t % HSG == HSG - 1:
                ptt = ptp.tile([128, 128], F32, tag="pt")
                nc.tensor.transpose(ptt[:], hs_cur[:], ident[:])
                out_cur = outp.tile([128, H], F32, tag="outstg")
                nc.vector.tensor_scalar_mul(out_cur[:], ptt[:], 0.5)
                dst = hs_d[:, t - HSG + 1:t + 1, :].rearrange("b t h -> t b h")
                nc.sync.dma_start(dst, out_cur[:])

            if tl == CHUNK - 1 and ch + 1 < nchunks:
                pg_cur = pg_next

    nc.finalize()
    return nc


def _pack_weights(W_ih, W_hh, b_ih, b_hh):
    Wh = W_hh.reshape(4, H, H).astype(np.float64)
    Wi = W_ih.reshape(4, H, I_DIM).astype(np.float64)
    bb = (b_ih.astype(np.float64) + b_hh.astype(np.float64)).reshape(4, H)
    rw = np.zeros((H, 4 * H), np.float64)        # [k, (s m)]
    wi = np.zeros((I_DIM + 1, 4 * H), np.float64)
    for s in range(4):
        g = GATE_PERM[s]
        k = GATE_SCALE[s]
        rw[:, s * H:(s + 1) * H] = (0.5 * k * Wh[g]).T
        wi[0:I_DIM, s * H:(s + 1) * H] = (k * Wi[g]).T
        wi[I_DIM, s * H:(s + 1) * H] = k * bb[g]
    import ml_dtypes
    return (rw.astype(ml_dtypes.bfloat16), wi.astype(np.float32))


_CACHE = {}


def _get_nc(T):
    if T not in _CACHE:
        _CACHE[T] = _build(T)
    return _CACHE[T]


def kernel(input_data, h0, c0, W_ih, W_hh, b_ih, b_hh):
    input_data = np.asarray(input_data, np.float32)
    h0 = np.asarray(h0, np.float32)
    c0 = np.asarray(c0, np.float32)
    T = input_data.shape[1]
    rw, wi = _pack_weights(np.asarray(W_ih, np.float32),
                           np.asarray(W_hh, np.float32),
                           np.asarray(b_ih, np.float32),
                           np.asarray(b_hh, np.float32))
    ident = np.eye(128, dtype=np.float32)
    nc = _get_nc(T)
    in_maps = []
    for c in range(NCORES):
        sl = slice(c * BL, (c + 1) * BL)
        in_maps.append({
            "x": np.ascontiguousarray(input_data[sl]),
            "h2": np.ascontiguousarray(2.0 * h0[sl]),
            "s": np.ascontiguousarray(2.0 * c0[sl]),
            "rw": rw, "wi": wi, "ident": ident,
        })
    res = run_bass_kernel_spmd(nc, in_maps, core_ids=list(range(NCORES)))
    hs = np.concatenate([res.results[c]["hs"] for c in range(NCORES)], axis=0)
    h_last = hs[:, -1, :][None]
    return (h_last, hs)


# revision 15
# speedup vs baseline: 1.4824x; 1.4824x over previous
"""LSTM encoder (B=128, T=2048, I=64, H=128) on 8 Trainium2 NeuronCores.

Strategy: data-parallel over batch (16 rows/core). Each core runs the full
sequential recurrence with a transposed layout ([H, b] tiles) so the gate
matmuls use the full 128-partition contraction and the elementwise phase
uses all 128 lanes.

Math (per step, PyTorch gate order i,f,g,o):
    gates = x W_ih^T + b_ih + b_hh + h W_hh^T
    c' = sigmoid(f) c + sigmoid(i) tanh(g);  h' = sigmoid(o) tanh(c')

On-chip algebra uses tanh only, with sigma(x) = (tanh(x/2)+1)/2 folded into
weight prescaling. State kept as S = 2c, H2 = 2h:
    PSUM slot s in {i,f,o}: a_s/2 ; slot g: a_g     (a_* = true preactivations)
    t* = tanh(PSUM)                                  -> ACT1, one instr
    u  = (t_i + 1) * t_g                             -> DVE stt
    t1 = (t_f + 1) * S                               -> DVE stt (fused w/ u)
    S' = 0.5 t1 + u                                  -> DVE stt
    y  = tanh(0.5 S')                                -> ACT2
    H2' = (t_o + 1) * y                              -> DVE stt
The input projections for a 16-step chunk are matmul'd into PSUM first
(start=True); the per-step recurrence matmuls accumulate onto them
(start=False), so no separate gate add is needed.
"""

import numpy as np
from contextlib import ExitStack

import concourse.bass as bass
import concourse.tile as tile
from concourse import bacc, mybir
from concourse.bass_utils import run_bass_kernel_spmd

F32 = mybir.dt.float32
BF16 = mybir.dt.bfloat16
AF = mybir.ActivationFunctionType
OP = mybir.AluOpType

B, T_FULL, I_DIM, H = 128, 2048, 64, 128
NCORES = 8
BL = B // NCORES            # 16 batch rows per core

# PSUM slot order (i, f, o, g); index into PyTorch gate order (i, f, g, o)
GATE_PERM = [0, 1, 3, 2]
GATE_SCALE = [0.5, 0.5, 0.5, 1.0]

CHUNK = 16                  # timesteps of input projection per PSUM chunk
HSG = 8                     # steps per hs staging group (PE transpose granularity)
OUTG = 64                   # steps per output DMA


def _build(T, features="rxo", t_mod=None):
    tm = t_mod or T
    assert T % OUTG == 0 and T % CHUNK == 0
    nchunks = T // CHUNK
    nc = bacc.Bacc("TRN2", target_bir_lowering=False, debug=False,
                   num_devices=NCORES)

    x_d = nc.dram_tensor("x", [BL, tm, I_DIM], F32, kind="ExternalInput")
    h2_d = nc.dram_tensor("h2", [BL, H], F32, kind="ExternalInput")   # 2*h0
    s_d = nc.dram_tensor("s", [BL, H], F32, kind="ExternalInput")     # 2*c0
    rw_d = nc.dram_tensor("rw", [H, 4 * H], BF16, kind="ExternalInput")
    wi_d = nc.dram_tensor("wi", [I_DIM + 1, 4 * H], F32, kind="ExternalInput")
    id_d = nc.dram_tensor("ident", [128, 128], F32, kind="ExternalInput")
    hs_d = nc.dram_tensor("hs", [BL, tm, H], F32, kind="ExternalOutput")

    with tile.TileContext(nc) as tc, ExitStack() as ctx:
        const = ctx.enter_context(tc.tile_pool(name="const", bufs=1))
        actsp = ctx.enter_context(tc.tile_pool(name="acts", bufs=4))
        xap = ctx.enter_context(tc.tile_pool(name="xa", bufs=1))
        h2p = ctx.enter_context(tc.tile_pool(name="h2b", bufs=8))
        scrp = ctx.enter_context(tc.tile_pool(name="scr", bufs=4))
        hsp = ctx.enter_context(tc.tile_pool(name="hsstg", bufs=2))
        outp = ctx.enter_context(tc.tile_pool(name="outstg", bufs=3))
        xinp = ctx.enter_context(tc.tile_pool(name="xin", bufs=3))
        pgp = ctx.enter_context(tc.tile_pool(name="pg", bufs=2, space="PSUM"))
        ptp = ctx.enter_context(tc.tile_pool(name="pt", bufs=2, space="PSUM"))

        rw_sb = const.tile([H, 4 * H], BF16)
        wi_sb = const.tile([I_DIM + 1, 4 * H], F32)
        ident = const.tile([128, 128], F32)
        nc.sync.dma_start(rw_sb[:], rw_d[:])
        nc.sync.dma_start(wi_sb[:], wi_d[:])
        nc.sync.dma_start(ident[:], id_d[:])

        # acts tile columns: 0:64 tanh gates (i,f,o,g), 64:80 S, 80:96 y.
        # Rotated every step; S' of step t is written into step t+1's tile.
        P_cur = actsp.tile([128, 96], F32, tag="acts", name="acts_init")
        # Two persistent x-hat buffers [I+1, CHUNK*BL]; row 64 = ones (bias).
        xa_bufs = [xap.tile([I_DIM + 1, CHUNK * BL], F32, tag=f"xa{i}",
                            name=f"xa{i}")
                   for i in range(2)]
        for xb in xa_bufs:
            nc.vector.memset(xb[I_DIM:I_DIM + 1, :], 1.0)

        # ---- initial state ----
        init_in = xinp.tile([128, 128], F32, tag="init")
        nc.sync.dma_start(init_in[0:BL, :], h2_d[:])
        init_in2 = xinp.tile([128, 128], F32, tag="init")
        nc.sync.dma_start(init_in2[0:BL, :], s_d[:])
        pt0 = ptp.tile([128, 128], F32, tag="pt")
        nc.tensor.transpose(pt0[:, 0:BL], init_in[0:BL, :], ident[0:BL, 0:BL])
        h2b_prev = h2p.tile([128, BL], BF16)
        nc.vector.tensor_copy(h2b_prev[:], pt0[:, 0:BL])
        pt1 = ptp.tile([128, 128], F32, tag="pt")
        nc.tensor.transpose(pt1[:, 0:BL], init_in2[0:BL, :], ident[0:BL, 0:BL])
        nc.vector.tensor_copy(P_cur[:, 64:80], pt1[:, 0:BL])

        def emit_xhat(ch, half):
            """Load+transpose 8 steps of x into xa buffer for chunk ch."""
            t0 = (ch * CHUNK + half * 8) % tm
            xa = xa_bufs[ch % 2]
            xin = xinp.tile([128, I_DIM], F32)
            src = x_d[:, t0:t0 + 8, :].rearrange("b t i -> t b i")
            nc.sync.dma_start(xin[:], src)
            pt = ptp.tile([128, 128], F32, tag="pt")
            nc.tensor.transpose(pt[0:I_DIM, :], xin[:], ident[:])
            nc.vector.tensor_copy(
                xa[0:I_DIM, half * 128:(half + 1) * 128], pt[0:I_DIM, 0:128])

        def emit_xg(ch, pg, half):
            """Input-projection matmuls for 8 steps into PSUM chunk."""
            xa = xa_bufs[ch % 2]
            v = pg[:].rearrange("p (t s b) -> p t s b", t=CHUNK, s=4)
            for s in range(4):
                # start=True clears the has_written bits of the WHOLE bank,
                # so only the first matmul into each bank may set it.
                nc.tensor.matmul(
                    v[:, half * 8:(half + 1) * 8, s, :],
                    wi_sb[:, s * H:(s + 1) * H],
                    xa[:, half * 128:(half + 1) * 128],
                    start=(s == 0), stop=False, skip_group_check=True)

        # chunk 0 production
        pg_cur = pgp.tile([128, CHUNK * 64], F32, tag="pg")
        if "x" in features:
            emit_xhat(0, 0)
            emit_xhat(0, 1)
            emit_xg(0, pg_cur, 0)
            emit_xg(0, pg_cur, 1)
        pg_next = None

        hs_cur = None
        out_cur = None
        for t in range(T):
            ch, tl = divmod(t, CHUNK)
            # interleave next-chunk production
            if "x" in features and ch + 1 < nchunks:
                if tl == 0:
                    pg_next = pgp.tile([128, CHUNK * 64], F32, tag="pg")
                    emit_xhat(ch + 1, 0)
                elif tl == 2:
                    emit_xhat(ch + 1, 1)
                elif tl == 8:
                    emit_xg(ch + 1, pg_next, 0)
                elif tl == 10:
                    emit_xg(ch + 1, pg_next, 1)

            # ---- recurrence step ----
            P_next = actsp.tile([128, 96], F32, tag="acts", name="acts_t")
            vg = pg_cur[:, tl * 64:(tl + 1) * 64]
            for s in range(4):
                nc.tensor.matmul(
                    vg[:, s * BL:(s + 1) * BL],
                    rw_sb[:, s * H:(s + 1) * H],
                    h2b_prev[:],
                    start=("x" not in features and s == 0),
                    stop=True, skip_group_check=True)
            nc.scalar.activation(P_cur[:, 0:64], vg[:], AF.Tanh)
            scr = scrp.tile([128, 2 * BL], F32)
            # [u | t1] = (([t_i|t_f]) + 1) * ([t_g | S])
            nc.vector.scalar_tensor_tensor(
                scr[:], P_cur[:, 0:32], 1.0, P_cur[:, 48:80], OP.add, OP.mult)
            # S' = 0.5*t1 + u  (into next step's tile)
            nc.vector.scalar_tensor_tensor(
                P_next[:, 64:80], scr[:, BL:2 * BL], 0.5, scr[:, 0:BL],
                OP.mult, OP.add)
            nc.scalar.activation(P_cur[:, 80:96], P_next[:, 64:80], AF.Tanh,
                                 scale=0.5)
            h2b_new = h2p.tile([128, BL], BF16)
            nc.vector.scalar_tensor_tensor(
                h2b_new[:], P_cur[:, 32:48], 1.0, P_cur[:, 80:96],
                OP.add, OP.mult)
            if "o" in features:
                if t % HSG == 0:
                    hs_cur = hsp.tile([128, HSG * BL], F32)
                nc.vector.scalar_tensor_tensor(
                    hs_cur[:, (t % HSG) * BL:(t % HSG + 1) * BL],
                    P_cur[:, 32:48], 1.0, P_cur[:, 80:96], OP.add, OP.mult)
            h2b_prev = h2b_new
            P_cur = P_next

            if "o" in features and t % HSG == HSG - 1:
                ptt = ptp.tile([128, 128], F32, tag="pt")
                nc.tensor.transpose(ptt[:], hs_cur[:], ident[:])
                out_cur = outp.tile([128, H], F32, tag="outstg")
                nc.vector.tensor_scalar_mul(out_cur[:], ptt[:], 0.5)
                tw = (t - HSG + 1) % tm
                dst = hs_d[:, tw:tw + HSG, :].rearrange("b t h -> t b h")
                nc.sync.dma_start(dst, out_cur[:])

            if tl == CHUNK - 1 and ch + 1 < nchunks:
                if pg_next is None:
                    pg_next = pgp.tile([128, CHUNK * 64], F32, tag="pg")
                pg_cur = pg_next
                pg_next = None

    nc.finalize()
    return nc


def _pack_weights(W_ih, W_hh, b_ih, b_hh):
    Wh = W_hh.reshape(4, H, H).astype(np.float64)
    Wi = W_ih.reshape(4, H, I_DIM).astype(np.float64)
    bb = (b_ih.astype(np.float64) + b_hh.astype(np.float64)).reshape(4, H)
    rw = np.zeros((H, 4 * H), np.float64)        # [k, (s m)]
    wi = np.zeros((I_DIM + 1, 4 * H), np.float64)
    for s in range(4):
        g = GATE_PERM[s]
        k = GATE_SCALE[s]
        rw[:, s * H:(s + 1) * H] = (0.5 * k * Wh[g]).T
        wi[0:I_DIM, s * H:(s + 1) * H] = (k * Wi[g]).T
        wi[I_DIM, s * H:(s + 1) * H] = k * bb[g]
    import ml_dtypes
    return (rw.astype(ml_dtypes.bfloat16), wi.astype(np.float32))


_CACHE = {}


def _get_nc(T, features="rxo", t_mod=None):
    key = (T, features, t_mod)
    if key not in _CACHE:
        _CACHE[key] = _build(T, features, t_mod)
    return _CACHE[key]


def kernel(input_data, h0, c0, W_ih, W_hh, b_ih, b_hh):
    input_data = np.asarray(input_data, np.float32)
    h0 = np.asarray(h0, np.float32)
    c0 = np.asarray(c0, np.float32)
    T = input_data.shape[1]
    rw, wi = _pack_weights(np.asarray(W_ih, np.float32),
                           np.asarray(W_hh, np.float32),
                           np.asarray(b_ih, np.float32),
                           np.asarray(b_hh, np.float32))
    ident = np.eye(128, dtype=np.float32)
    nc = _get_nc(T)
    in_maps = []
    for c in range(NCORES):
        sl = slice(c * BL, (c + 1) * BL)
        in_maps.append({
            "x": np.ascontiguousarray(input_data[sl]),
            "h2": np.ascontiguousarray(2.0 * h0[sl]),
            "s": np.ascontiguousarray(2.0 * c0[sl]),
            "rw": rw, "wi": wi, "ident": ident,
        })
    res = run_bass_kernel_spmd(nc, in_maps, core_ids=list(range(NCORES)))
    hs = np.concatenate([res.results[c]["hs"] for c in range(NCORES)], axis=0)
    h_last = hs[:, -1, :][None]
    return (h_last, hs)
